# revision 1
# baseline (speedup 1.0000x reference)
"""Trainium2 8-core tensor-parallel attention kernel (Bass/Tile).

Full inputs in, full output out. Sharding: tensor-parallel over heads
(4 heads per core), AllGather of attention outputs, each core computes a
512-wide output-column slice of the o_proj; host concatenates.
"""
import sys

for _p in ("/opt/trn_rl_repo",):
    if _p not in sys.path:
        sys.path.insert(0, _p)

import numpy as np
import ml_dtypes

import concourse.bass as bass
import concourse.mybir as mybir
import concourse.tile as tile
from concourse import bacc
from concourse.bass_utils import run_bass_kernel_spmd

B, S, D, H = 2, 2048, 4096, 32
HD = D // H          # 128 head dim
T = B * S            # 4096 tokens
NC = 8               # cores
HL = H // NC         # 4 heads per core
DH = HL * HD         # 512 dims per core
SCALE = 1.0 / float(np.sqrt(HD))
BF16 = mybir.dt.bfloat16
F32 = mybir.dt.float32
bf16 = ml_dtypes.bfloat16

_CACHE = {}
LAST_RESULT = None


def build():
    nc = bacc.Bacc("TRN2", target_bir_lowering=False, debug=False, num_devices=NC)

    xT = nc.dram_tensor("xT", [D, T], BF16, kind="ExternalInput").ap()
    wqT = nc.dram_tensor("wqT", [D, DH], BF16, kind="ExternalInput").ap()
    wkT = nc.dram_tensor("wkT", [D, DH], BF16, kind="ExternalInput").ap()
    wvT = nc.dram_tensor("wvT", [D, DH], BF16, kind="ExternalInput").ap()
    woT = nc.dram_tensor("woT", [D, DH], BF16, kind="ExternalInput").ap()
    cosE = nc.dram_tensor("cosE", [HD, T], BF16, kind="ExternalInput").ap()
    sinE = nc.dram_tensor("sinE", [HD, T], BF16, kind="ExternalInput").ap()
    rotT = nc.dram_tensor("rotT", [HD, HD], BF16, kind="ExternalInput").ap()
    maskd = nc.dram_tensor("maskd", [16 * 128, 512], F32, kind="ExternalInput").ap()
    ones128 = nc.dram_tensor("ones128", [128, 1], BF16, kind="ExternalInput").ap()
    ones1f = nc.dram_tensor("ones1f", [1, 128], F32, kind="ExternalInput").ap()
    out = nc.dram_tensor("out", [T, DH], F32, kind="ExternalOutput").ap()

    NT = T // 512      # 8 token slices of 512
    NCT = D // 128     # 32 contraction tiles

    with tile.TileContext(nc) as tc:
        with tc.tile_pool(name="dram", bufs=1, space="DRAM") as dram:
            qTd = dram.tile([DH, T], BF16)
            kTd = dram.tile([DH, T], BF16)
            vd = dram.tile([T, DH], BF16)
            agin0 = dram.tile([DH, S], BF16)
            agin1 = dram.tile([DH, S], BF16)
            agout0 = dram.tile([NC * DH, S], BF16, addr_space="Shared")
            agout1 = dram.tile([NC * DH, S], BF16, addr_space="Shared")

            # ---------------- phase 1: QKV projections + RoPE ----------------
            with tc.tile_pool(name="wres", bufs=1) as wres, \
                 tc.tile_pool(name="xs", bufs=36) as xs, \
                 tc.tile_pool(name="rw", bufs=4) as rw, \
                 tc.tile_pool(name="qf", bufs=4) as qfp, \
                 tc.tile_pool(name="pp", bufs=2, space="PSUM") as pp:
                cos_sb = wres.tile([128, T], BF16, name="cos_sb")
                sin_sb = wres.tile([128, T], BF16, name="sin_sb")
                rot_sb = wres.tile([128, 128], BF16, name="rot_sb")
                nc.sync.dma_start(cos_sb[:], cosE[:])
                nc.sync.dma_start(sin_sb[:], sinE[:])
                nc.sync.dma_start(rot_sb[:], rotT[:])
                wtiles = {}
                for wname, w_dr in (("q", wqT), ("k", wkT), ("v", wvT)):
                    for c in range(NCT):
                        wt = wres.tile([128, DH], BF16, name=f"w{wname}_{c}")
                        nc.sync.dma_start(wt[:], w_dr[c * 128:(c + 1) * 128, :])
                        wtiles[(wname, c)] = wt

                for t in range(NT):
                    tok = t * 512
                    xt = []
                    for c in range(NCT):
                        xc = xs.tile([128, 512], BF16, tag="xt", name=f"x_{t}_{c}")
                        nc.sync.dma_start(xc[:], xT[c * 128:(c + 1) * 128, tok:tok + 512])
                        xt.append(xc)
                    # q/k projections (dim-major) + rope
                    for wname, dst in (("q", qTd), ("k", kTd)):
                        for i in range(HL):
                            ps = pp.tile([128, 512], F32, tag="ps", name=f"ps_{t}_{i}")
                            for c in range(NCT):
                                nc.tensor.matmul(
                                    ps[:], wtiles[(wname, c)][:, i * 128:(i + 1) * 128],
                                    xt[c][:], start=(c == 0), stop=(c == NCT - 1))
                            qsb = rw.tile([128, 512], BF16, tag="qsb", name=f"qsb{t}{i}")
                            nc.vector.tensor_copy(qsb[:], ps[:])
                            pr = pp.tile([128, 512], F32, tag="pr", name=f"pr_{t}_{i}")
                            nc.tensor.matmul(pr[:], rot_sb[:], qsb[:], start=True, stop=True)
                            qc = rw.tile([128, 512], BF16, tag="qc", name=f"qc{t}{i}")
                            nc.vector.tensor_tensor(qc[:], ps[:], cos_sb[:, tok:tok + 512], mybir.AluOpType.mult)
                            qr = rw.tile([128, 512], BF16, tag="qr", name=f"qr{t}{i}")
                            nc.vector.tensor_tensor(qr[:], pr[:], sin_sb[:, tok:tok + 512], mybir.AluOpType.mult)
                            qfin = qfp.tile([128, 512], BF16, tag="qfin", name=f"qf{t}{i}")
                            nc.vector.tensor_tensor(qfin[:], qc[:], qr[:], mybir.AluOpType.add)
                            nc.sync.dma_start(dst[i * 128:(i + 1) * 128, tok:tok + 512], qfin[:])
                    # v projection (token-major)
                    for tt in range(4):
                        psv = pp.tile([128, 512], F32, tag="psv", name=f"psv_{t}_{tt}")
                        for c in range(NCT):
                            nc.tensor.matmul(
                                psv[:], xt[c][:, tt * 128:(tt + 1) * 128],
                                wtiles[("v", c)][:],
                                start=(c == 0), stop=(c == NCT - 1))
                        vsb = qfp.tile([128, 512], BF16, tag="vsb", name=f"vsb{t}{tt}")
                        nc.vector.tensor_copy(vsb[:], psv[:])
                        nc.sync.dma_start(vd[tok + tt * 128: tok + (tt + 1) * 128, :], vsb[:])

            # ---------------- phase 2: attention ----------------
            with tc.tile_pool(name="ares", bufs=1) as ares, \
                 tc.tile_pool(name="ah", bufs=3) as ah, \
                 tc.tile_pool(name="aw", bufs=8) as aw, \
                 tc.tile_pool(name="aps", bufs=2, space="PSUM") as aps:
                maskd_sb = ares.tile([128, 16 * 512], F32, name="maskd_sb")
                nc.sync.dma_start(
                    maskd_sb[:].rearrange("p (blk q) -> p blk q", blk=16),
                    maskd.rearrange("(blk p) q -> p blk q", p=128))
                o128_sb = ares.tile([128, 1], BF16, name="o128_sb")
                nc.sync.dma_start(o128_sb[:], ones128[:])
                o1f_sb = ares.tile([1, 128], F32, name="o1f_sb")
                nc.sync.dma_start(o1f_sb[:], ones1f[:])

                for b in range(B):
                    for h in range(HL):
                        qh = ah.tile([128, S], BF16, tag="qh", name=f"qh{b}{h}")
                        kh = ah.tile([128, S], BF16, tag="kh", name=f"kh{b}{h}")
                        vh = ah.tile([128, 16 * 128], BF16, tag="vh", name=f"vh{b}{h}")
                        nc.sync.dma_start(qh[:], qTd[h * 128:(h + 1) * 128, b * S:(b + 1) * S])
                        nc.sync.dma_start(kh[:], kTd[h * 128:(h + 1) * 128, b * S:(b + 1) * S])
                        nc.sync.dma_start(
                            vh[:].rearrange("p (kt d) -> p kt d", kt=16),
                            vd.rearrange("(bb kt p) i -> bb p kt i", bb=B, p=128)[b, :, :, h * 128:(h + 1) * 128])
                        for jq in range(4):
                            nkt = 4 * (jq + 1)
                            acc = aps.tile([128, 512], F32, tag="acc", name=f"acc{b}{h}{jq}")
                            sums = aps.tile([1, 512], F32, tag="sums", name=f"sums{b}{h}{jq}")
                            for kt in range(nkt):
                                pss = aps.tile([128, 512], F32, tag="pss", name=f"pss{b}{h}{jq}{kt}")
                                nc.tensor.matmul(
                                    pss[:], kh[:, kt * 128:(kt + 1) * 128],
                                    qh[:, jq * 512:(jq + 1) * 512], start=True, stop=True)
                                if kt >= 4 * jq:
                                    blk = 4 * jq + (kt - 4 * jq)
                                    nc.vector.tensor_tensor(
                                        pss[:], pss[:],
                                        maskd_sb[:, blk * 512:(blk + 1) * 512],
                                        mybir.AluOpType.add)
                                ex = aw.tile([128, 512], BF16, tag="ex", name=f"ex{b}{h}{jq}{kt}")
                                nc.scalar.activation(ex[:], pss[:], mybir.ActivationFunctionType.Exp, scale=SCALE)
                                nc.tensor.matmul(acc[:], vh[:, kt * 128:(kt + 1) * 128], ex[:],
                                                 start=(kt == 0), stop=(kt == nkt - 1))
                                nc.tensor.matmul(sums[:], o128_sb[:], ex[:],
                                                 start=(kt == 0), stop=(kt == nkt - 1))
                            rec = aw.tile([1, 512], F32, tag="rec", name=f"rec{b}{h}{jq}")
                            nc.vector.reciprocal(rec[:], sums[:])
                            rb = aps.tile([128, 512], F32, tag="rb", name=f"rb{b}{h}{jq}")
                            nc.tensor.matmul(rb[:], o1f_sb[:], rec[:], start=True, stop=True)
                            rbs = aw.tile([128, 512], F32, tag="rbs", name=f"rbs{b}{h}{jq}")
                            nc.vector.tensor_copy(rbs[:], rb[:])
                            att = aw.tile([128, 512], BF16, tag="att", name=f"att{b}{h}{jq}")
                            nc.vector.tensor_tensor(att[:], acc[:], rbs[:], mybir.AluOpType.mult)
                            agin_b = agin0 if b == 0 else agin1
                            nc.sync.dma_start(
                                agin_b[h * 128:(h + 1) * 128, jq * 512:(jq + 1) * 512],
                                att[:])

            # ---------------- all-gathers (one per batch half) ----------------
            nc.gpsimd.collective_compute(
                "AllGather", mybir.AluOpType.bypass,
                replica_groups=[list(range(NC))],
                ins=[agin0.opt()], outs=[agout0.opt()])
            nc.gpsimd.collective_compute(
                "AllGather", mybir.AluOpType.bypass,
                replica_groups=[list(range(NC))],
                ins=[agin1.opt()], outs=[agout1.opt()])

            # ---------------- phase 3: o_proj ----------------
            with tc.tile_pool(name="ores", bufs=1) as ores, \
                 tc.tile_pool(name="och", bufs=3) as och, \
                 tc.tile_pool(name="oo", bufs=4) as oo, \
                 tc.tile_pool(name="ops", bufs=4, space="PSUM") as ops:
                wo_sb = ores.tile([128, NCT * DH], BF16, name="wo_sb")
                nc.sync.dma_start(
                    wo_sb[:].rearrange("p (c i) -> p c i", c=NCT),
                    woT.rearrange("(c p) i -> p c i", p=128))
                for t in range(T // 128):
                    agout_b = agout0 if t < 16 else agout1
                    tl = t % 16
                    ch = och.tile([128, NCT * 128], BF16, tag="ch", name=f"ch{t}")
                    nc.sync.dma_start(
                        ch[:].rearrange("p (c u) -> p c u", c=NCT),
                        agout_b.rearrange("(c p) t -> p c t", p=128)[:, :, tl * 128:(tl + 1) * 128])
                    pso = ops.tile([128, 512], F32, tag="pso", name=f"pso{t}")
                    for i in range(NCT):
                        nc.tensor.matmul(pso[:], ch[:, i * 128:(i + 1) * 128],
                                         wo_sb[:, i * DH:(i + 1) * DH],
                                         start=(i == 0), stop=(i == NCT - 1))
                    ot = oo.tile([128, 512], F32, tag="ot", name=f"ot{t}")
                    nc.vector.tensor_copy(ot[:], pso[:])
                    nc.sync.dma_start(out[t * 128:(t + 1) * 128, :], ot[:])

    nc.compile()
    return nc


def _host_prep(x, freqs_cos, freqs_sin, mask, wq, wk, wv, wo):
    xT = np.ascontiguousarray(x.reshape(T, D).T).astype(bf16)
    cos = np.asarray(freqs_cos, np.float32)   # [S, 64]
    sin = np.asarray(freqs_sin, np.float32)
    cosE = np.repeat(cos.T, 2, axis=0)        # [128, S]
    sinE = np.repeat(sin.T, 2, axis=0)
    cosE = np.tile(cosE, (1, B)).astype(bf16)  # [128, T] (b-major tokens)
    sinE = np.tile(sinE, (1, B)).astype(bf16)
    rot = np.zeros((HD, HD), np.float32)
    idx = np.arange(0, HD, 2)
    rot[idx, idx + 1] = -1.0                  # rot(q)[2d] = -q[2d+1]
    rot[idx + 1, idx] = 1.0                   # rot(q)[2d+1] = q[2d]
    rotT = np.ascontiguousarray(rot.T).astype(bf16)
    maskT = np.asarray(mask, np.float32).T / SCALE
    maskd = np.zeros((16, 128, 512), np.float32)
    for jq in range(4):
        for kb in range(4):
            k0 = jq * 512 + kb * 128
            maskd[jq * 4 + kb] = maskT[k0:k0 + 128, jq * 512:(jq + 1) * 512]
    maskd = maskd.reshape(16 * 128, 512)
    ones128 = np.ones((128, 1), bf16)
    ones1f = np.ones((1, 128), np.float32)
    shared = dict(xT=xT, cosE=cosE, sinE=sinE, rotT=rotT, maskd=maskd,
                  ones128=ones128, ones1f=ones1f)
    in_maps = []
    for r in range(NC):
        sl = slice(r * DH, (r + 1) * DH)
        m = dict(shared)
        m["wqT"] = np.ascontiguousarray(np.asarray(wq, np.float32)[sl, :].T).astype(bf16)
        m["wkT"] = np.ascontiguousarray(np.asarray(wk, np.float32)[sl, :].T).astype(bf16)
        m["wvT"] = np.ascontiguousarray(np.asarray(wv, np.float32)[sl, :].T).astype(bf16)
        m["woT"] = np.ascontiguousarray(np.asarray(wo, np.float32)[sl, :].T).astype(bf16)
        in_maps.append(m)
    return in_maps


def kernel(x, freqs_cos, freqs_sin, mask, wq, wk, wv, wo, start_pos):
    global LAST_RESULT
    if "nc" not in _CACHE:
        _CACHE["nc"] = build()
    nc = _CACHE["nc"]
    in_maps = _host_prep(x, freqs_cos, freqs_sin, mask, wq, wk, wv, wo)
    res = run_bass_kernel_spmd(nc, in_maps, core_ids=list(range(NC)))
    LAST_RESULT = res
    parts = [res.results[r]["out"] for r in range(NC)]
    full = np.concatenate(parts, axis=1)      # [T, D]
    return np.ascontiguousarray(full.reshape(B, S, D)).astype(np.float32)

